# revision 1
# baseline (speedup 1.0000x reference)
"""Trainium2 Bass kernel for the ADMM total-variation solver (nn_ADMM).

Math: the reference iterates x <- resid @ inv(AtA + g*DtD + a*I) 50 times.
AtA is rank-9 (A is [9, 4096]) and C := g*DtD + a*I is a symmetric
tridiagonal circulant, so by Woodbury

    B^-1 = C^-1 - U S^-1 U^T,   U = C^-1 A^T,  S = I9 + A U

where C^-1 is a circulant whose kernel decays like 0.5^|d|.  The dense
4096x4096 matvec therefore becomes a banded (radius-32) circular
convolution plus a rank-9 correction - tiny enough to keep everything
resident in SBUF on a single NeuronCore with no HBM traffic inside the
iteration loop.  All 8 cores run the same program (SPMD, no collectives);
core 0's output is returned.

Device layout: vectors live as [128, 34] tiles with halo columns,
flat index i = k + 128*c stored at tile[:, c+1]; col 0 mirrors col 32
and col 33 mirrors col 1.  A banded circulant operator is then 2-3
matmuls: a [128,128] main stationary on cols 1:33 plus corner blocks
(padded to K=128) on the column-shifted views - the halo provides the
circular wrap for free.

Per iteration (scaled states Eb = eta, Tb = tau, Ub = g*u, Wb = a*w):
PE computes only the banded convolutions into PSUM (plus bf16 filler
matmuls that keep the HAM clock ramp at 2.4 GHz); cross-tensor adds and
the soft-threshold (z - clip(z,-lam,lam), two DVE ops) run on DVE; alpha*x
and the j=1 relu run on ACT; state prep and halo copies on GpSimd.
The rank-9 term q = U^T v is a DVE multiply+reduce, and a single
all-ones [128,128] matmul performs the partition reduction AND the
broadcast of q in one shot.
"""

import numpy as np

N = 4096
P = 128          # partitions
CCOL = 32        # payload columns; i = k + 128*c at tile col c+1
HCOL = CCOL + 2  # halo tile width
RB = 32          # band radius
RBLK = 64        # corner block active rows
R9 = 9           # Woodbury rank
GAMMA = 10.0
ALPHA = 5.0
LAM = 1e-4
NIT = 50
NCORES = 8

# column offsets inside the constant blob [128, BLOB_COLS]
OFF_OPA1_M = 0       # gamma*(S+ - I) main
OFF_OPB_M = 128      # gamma*(S- - I) main
OFF_G_M = 256        # C^-1 banded main
OFF_AI = 384         # alpha*I
OFF_ONESF = 512      # all-ones (partition reduce + broadcast)
OFF_OPA1_BL = 640    # corner blocks, K padded to 128
OFF_OPB_BH = 768
OFF_G_BL = 896
OFF_G_BH = 1024
OFF_USTACK2 = 1152   # U, m-major: [k, m*32+c]
OFF_W2STACK = 1440   # W2, c-major: [k, c*9+m]
OFF_C0 = 1728        # B^-1 bA tile
OFF_X0 = 1760        # x0 tile WITH halo columns [128, 34]
OFF_MLAM = 1794      # column of -LAM (activation bias AP)
OFF_I = 1824         # identity (c0 accumulate pair in the E group)
BLOB_COLS = 1952


def _banded_mats(h):
    """lhsT pieces for kernel h (dict d -> coef), all [*, 128] columns.
    main: within-column;  BL: reads column c-1 (rows 64..127 active);
    BH: reads column c+1 (rows 0..63 active).  Corner arrays are
    returned [128, 128] with inactive rows zero (K padded to 128 so all
    matmuls share tile_position (0, 0))."""
    B0 = np.zeros((P, P), np.float64)
    BL = np.zeros((P, P), np.float64)
    BH = np.zeros((P, P), np.float64)
    for k in range(P):
        for m in range(P):
            d = k - m
            if d in h:
                B0[k, m] = h[d]
    for a in range(RBLK):
        for m in range(P):
            d = (a + P - RBLK) - P - m      # BL active row k' = 64 + a
            if d in h:
                BL[a + P - RBLK, m] = h[d]
            d = a + P - m                   # BH active row k' = a
            if d in h:
                BH[a, m] = h[d]
    return B0, BL, BH


def _tile(vec):
    """[4096] -> [128, 32], i = k + 128*c."""
    return np.ascontiguousarray(vec.reshape(CCOL, P).T)


def host_constants(target, A, x0):
    """All f64 precompute; returns the [128, BLOB_COLS] f32 device blob."""
    A64 = np.asarray(A, np.float64)
    w = ALPHA + 2 * GAMMA * (1 - np.cos(2 * np.pi * np.arange(N // 2 + 1) / N))

    def C_inv(z):
        return np.fft.irfft(np.fft.rfft(z, axis=-1) / w, n=N, axis=-1)

    U = C_inv(A64).T                              # [N, 9]
    S = np.eye(R9) + A64 @ U
    W2 = U @ np.linalg.inv(S)                     # [N, 9]
    g = np.fft.irfft(1.0 / w, n=N)                # kernel of C^-1
    b = A64 @ np.asarray(target, np.float64)
    bA = b @ A64
    c0 = C_inv(bA) - W2 @ (U.T @ bA)              # B^-1 @ bA

    blob = np.zeros((P, BLOB_COLS), np.float64)

    def put(off, arr):
        blob[:, off:off + arr.shape[1]] = arr

    mA1 = _banded_mats({-1: GAMMA, 0: -GAMMA})    # gamma*(S+ - I)
    mB = _banded_mats({1: GAMMA, 0: -GAMMA})      # gamma*(S- - I)
    mG = _banded_mats({d: g[d % N] for d in range(-RB, RB + 1)})
    put(OFF_OPA1_M, mA1[0]); put(OFF_OPA1_BL, mA1[1])
    put(OFF_OPB_M, mB[0]); put(OFF_OPB_BH, mB[2])
    put(OFF_G_M, mG[0]); put(OFF_G_BL, mG[1]); put(OFF_G_BH, mG[2])
    put(OFF_AI, ALPHA * np.eye(P))
    put(OFF_ONESF, np.ones((P, P)))

    # Ustack2[k, m*32+c] = U[k + 128c, m]  (m-major, for q = U^T v)
    put(OFF_USTACK2, U.reshape(CCOL, P, R9).transpose(1, 2, 0).reshape(P, R9 * CCOL))
    # W2stack[k, c*9+m] = W2[k + 128c, m]  (c-major, for corr = W2 q)
    put(OFF_W2STACK, W2.reshape(CCOL, P, R9).transpose(1, 0, 2).reshape(P, CCOL * R9))
    put(OFF_C0, _tile(c0))
    put(OFF_I, np.eye(P))

    x0t = _tile(np.asarray(x0, np.float64))
    blob[:, OFF_X0 + 1:OFF_X0 + 1 + CCOL] = x0t
    blob[:, OFF_X0] = x0t[:, CCOL - 1]            # halo: col0 = col32
    blob[:, OFF_X0 + CCOL + 1] = x0t[:, 0]        # halo: col33 = col1
    blob[:, OFF_MLAM] = -LAM
    return np.ascontiguousarray(blob.astype(np.float32))


def build_nc():
    """Build and compile the Bacc graph (one core's program)."""
    from concourse import bacc, mybir, tile

    f32 = mybir.dt.float32
    Relu = mybir.ActivationFunctionType.Relu
    Alu = mybir.AluOpType
    nc = bacc.Bacc(target_bir_lowering=False)

    blob_ext = nc.declare_dram_parameter("blob", [P, BLOB_COLS], f32, isOutput=False)
    out_ext = nc.declare_dram_parameter("out", [P, CCOL], f32, isOutput=True)

    with tile.TileContext(nc) as tc:
        with (
            tc.tile_pool(name="const", bufs=1) as cpool,
            tc.tile_pool(name="work", bufs=3) as wpool,
            tc.tile_pool(name="psum", bufs=1, space="PSUM") as ppool,
        ):
            cb = cpool.tile([P, BLOB_COLS], f32, tag="blob")
            nc.sync.dma_start(cb[:, :], blob_ext[:, :])

            def cs(off, width):
                return cb[:, off:off + width]

            A1_M, A1_BL = cs(OFF_OPA1_M, P), cs(OFF_OPA1_BL, P)
            B_M, B_BH = cs(OFF_OPB_M, P), cs(OFF_OPB_BH, P)
            G_M, G_BL, G_BH = cs(OFF_G_M, P), cs(OFF_G_BL, P), cs(OFF_G_BH, P)
            Copy = mybir.ActivationFunctionType.Copy
            onesf = cs(OFF_ONESF, P)
            U2_3d = cs(OFF_USTACK2, R9 * CCOL).rearrange("k (m c) -> k m c", c=CCOL)
            W2_3d = cs(OFF_W2STACK, CCOL * R9).rearrange("k (c m) -> k c m", m=R9)
            c0_t = cs(OFF_C0, CCOL)
            I_t = cs(OFF_I, P)
            mlam = cb[:, OFF_MLAM:OFF_MLAM + 1]

            def emit_bank(mms):
                n = len(mms)
                for i, (o, l, r) in enumerate(mms):
                    nc.tensor.matmul(o, l, r, start=(i == 0), stop=(i == n - 1))

            def halo_fix(xh):
                """mirror payload edge columns into the halo columns"""
                nc.gpsimd.tensor_copy(xh[:, 0:1], xh[:, CCOL:CCOL + 1])
                nc.gpsimd.tensor_copy(xh[:, HCOL - 1:HCOL], xh[:, 1:2])

            # state (python refs); X is a halo tile view [128, 34]
            Xh = cs(OFF_X0, HCOL)
            E_sb = T_sb = U_sb = W_sb = None

            NWARM = 8
            bf16 = mybir.dt.bfloat16
            warm_w = cs(OFF_ONESF, P).bitcast(bf16)[:, 0:P]
            warm_x = cs(OFF_ONESF, P).bitcast(bf16)[:, 0:2 * P]

            for j in range(1, NIT + 1):
                first = (j == 1)
                second = (j == 2)

                # keep the PE clock warm with filler bf16 matmuls
                if NWARM:
                    scratch = ppool.tile([P, 2 * P], f32, tag="scratch")
                    for wi in range(NWARM):
                        nc.tensor.matmul(scratch[:, :], warm_w, warm_x,
                                         start=True, stop=True)

                bankA = ppool.tile([P, CCOL], f32, tag="A")
                bankE = ppool.tile([P, CCOL], f32, tag="E")
                bankR = ppool.tile([P, R9], f32, tag="R")
                if not first:
                    bankB = ppool.tile([P, CCOL], f32, tag="B")

                xm, xl, xr = Xh[:, 1:CCOL + 1], Xh[:, 0:CCOL], Xh[:, 2:HCOL]

                # early combos from previous state (GpSimd: off critical path)
                if not first:
                    tAB = wpool.tile([P, CCOL], f32, tag="tAB")
                    tTW = wpool.tile([P, CCOL], f32, tag="tTW")
                    if second:
                        nc.gpsimd.tensor_scalar_mul(tAB[:, :], U_sb[:, :], -1.0)
                        nc.gpsimd.tensor_scalar_mul(tTW[:, :], W_sb[:, :], -1.0)
                    else:
                        nc.gpsimd.tensor_sub(tAB[:, :], E_sb[:, :], U_sb[:, :])
                        nc.gpsimd.tensor_sub(tTW[:, :], T_sb[:, :], W_sb[:, :])

                # PE: banded mains only; wrap corners are single elements
                # for A1/B and get applied as GpSimd row fixes below
                if not first:
                    nc.tensor.matmul(bankB[:, :], B_M, xm, start=True, stop=False)
                    nc.tensor.matmul(bankA[:, :], A1_M, xm, start=True, stop=False)
                    nc.tensor.matmul(bankB[:, :], B_BH, xr, start=False, stop=True)
                    nc.tensor.matmul(bankA[:, :], A1_BL, xl, start=False, stop=True)
                else:
                    emit_bank([(bankA[:, :], A1_M, xm), (bankA[:, :], A1_BL, xl)])

                r1 = wpool.tile([P, CCOL], f32, tag="r1")
                Un = wpool.tile([P, CCOL], f32, tag="Un")
                Wn = wpool.tile([P, CCOL], f32, tag="Wn")
                vh = wpool.tile([P, HCOL], f32, tag="vh")
                vm, vl, vr = vh[:, 1:CCOL + 1], vh[:, 0:CCOL], vh[:, 2:HCOL]

                if first:
                    At_src = bankA[:, :]
                    En = Tn = None
                    # Wb1 = relu(alpha * x0)
                    nc.scalar.activation(Wn[:, :], xm, Relu, bias=0.0, scale=ALPHA)
                else:
                    Cx = wpool.tile([P, CCOL], f32, tag="Cx")
                    En = wpool.tile([P, CCOL], f32, tag="En")
                    Tn = wpool.tile([P, CCOL], f32, tag="Tn")
                    At = wpool.tile([P, CCOL], f32, tag="At")
                    Dt = wpool.tile([P, CCOL], f32, tag="Dt")
                    nc.scalar.activation(Cx[:, :], xm,
                                         mybir.ActivationFunctionType.Copy,
                                         bias=0.0, scale=ALPHA)
                    # DVE order fills the idle window between bankB-ready
                    # and bankA-ready with the Tn/P2n/Dt work
                    nc.vector.tensor_add(En[:, :], bankB[:, :], tAB[:, :])
                    nc.vector.tensor_add(Tn[:, :], Cx[:, :], tTW[:, :])
                    nc.vector.tensor_add(Dt[:, :], Cx[:, :], Tn[:, :])
                    P2n = wpool.tile([P, CCOL], f32, tag="P2n")
                    # P2n = -(En + Tn): ready before bankA lands
                    nc.vector.scalar_tensor_tensor(P2n[:, :], En[:, :], -1.0,
                                                   Tn[:, :], Alu.mult, Alu.subtract)
                    nc.vector.tensor_add(At[:, :], bankA[:, :], En[:, :])
                    At_src = At[:, :]
                # soft threshold on DVE in 2 ops: soft(z) = z - clip(z,-lam,lam)
                nc.vector.tensor_scalar(r1[:, :], At_src, -LAM, LAM,
                                        Alu.max, Alu.min)
                nc.vector.tensor_sub(Un[:, :], At_src, r1[:, :])
                if first:
                    nc.vector.tensor_add(vm, Un[:, :], Wn[:, :])
                else:
                    P1 = wpool.tile([P, CCOL], f32, tag="P1")
                    # P1 = relu(Dt) + Un in one op; Wn state off-path (GpSimd)
                    nc.vector.scalar_tensor_tensor(P1[:, :], Dt[:, :], 0.0,
                                                   Un[:, :], Alu.max, Alu.add)
                    nc.gpsimd.tensor_scalar_max(Wn[:, :], Dt[:, :], 0.0)
                    nc.vector.tensor_add(vm, P1[:, :], P2n[:, :])
                halo_fix(vh)

                # y = G v + c0 (PE).  Emitted in two pieces with the
                # rank-9 reduce/broadcast matmul (bankR) in between, so the
                # Z2 chain starts while the G corner matmuls still run.

                # rank-9: q = U^T v via DVE; all-ones matmul reduces over
                # partitions AND broadcasts q to [128, 9]
                Z1 = wpool.tile([P, R9 * CCOL], f32, tag="Z1")
                Z1r = wpool.tile([P, R9], f32, tag="Z1r")
                z1_3d = Z1[:, :].rearrange("k (m c) -> k m c", c=CCOL)
                vb = vm.unsqueeze(1).broadcast_to([P, R9, CCOL])
                nc.vector.tensor_mul(z1_3d, U2_3d, vb)
                nc.vector.tensor_reduce(Z1r[:, :], z1_3d, axis=mybir.AxisListType.X,
                                        op=Alu.add)
                nc.tensor.matmul(bankE[:, :], I_t, c0_t, start=True, stop=False)
                nc.tensor.matmul(bankE[:, :], G_M, vm, start=False, stop=False)
                emit_bank([(bankR[:, :], onesf, Z1r[:, :])])
                nc.tensor.matmul(bankE[:, :], G_BL, vl, start=False, stop=False)
                nc.tensor.matmul(bankE[:, :], G_BH, vr, start=False, stop=True)

                Z2 = wpool.tile([P, CCOL * R9], f32, tag="Z2")
                corr = wpool.tile([P, CCOL], f32, tag="corr")
                z2_3d = Z2[:, :].rearrange("k (c m) -> k c m", m=R9)
                rb = bankR[:, :].unsqueeze(1).broadcast_to([P, CCOL, R9])
                nc.vector.tensor_mul(z2_3d, W2_3d, rb)
                nc.vector.tensor_reduce(corr[:, :], z2_3d, axis=mybir.AxisListType.X,
                                        op=Alu.add)

                # x' = bankE - corr   (c0 accumulated into bankE on PE)
                Xn = wpool.tile([P, HCOL], f32, tag="Xh")
                nc.vector.tensor_sub(Xn[:, 1:CCOL + 1], bankE[:, :], corr[:, :])
                halo_fix(Xn)

                Xh, E_sb, T_sb, U_sb, W_sb = Xn, En, Tn, Un, Wn

            nc.sync.dma_start(out_ext[:, :], Xh[:, 1:CCOL + 1])

    nc.compile()
    return nc


def kernel(**inputs):
    from concourse.bass_utils import run_bass_kernel_spmd

    target = np.asarray(inputs["target"], np.float32)
    A = np.asarray(inputs["A"], np.float32)
    x0 = np.asarray(inputs["x0"], np.float32)

    blob = host_constants(target, A, x0)
    nc = build_nc()
    in_maps = [{"blob": blob} for _ in range(NCORES)]
    res = run_bass_kernel_spmd(nc, in_maps, core_ids=list(range(NCORES)))
    out_tile = np.asarray(res.results[0]["out"], np.float32)
    return np.ascontiguousarray(out_tile.T.reshape(-1))



# revision 8
# speedup vs baseline: 1.3610x; 1.3610x over previous
"""Trainium2 Bass kernel for the ADMM total-variation solver (nn_ADMM).

Math: x <- B^-1(bA + v) iterated 50x, B = AtA + g*DtD + a*I.  AtA is
rank-9 and C := g*DtD + a*I is circulant, so by Woodbury
    B^-1 = C^-1 - W2 U^T,   U = C^-1 A^T,  W2 = U S^-1,  S = I9 + A U.
C^-1 is applied as a banded (radius-32) circular convolution G; the
rank-9 correction uses q = U^T v.  All 8 cores run the same program
(SPMD, no collectives); core 0's output is returned.

Perf design (vs the fp32 baseline):
- every matmul is bf16 (4x PE throughput).  A1/B/I/ones have exact bf16
  entries; G is split hi+lo bf16 (two matmuls ~= fp16 operator
  precision), which kills the systematic operator-rounding error that a
  plain bf16 G accumulates over 50 non-contracting iterations.
- At = A1 x + B x + (E - U) is accumulated entirely in one PSUM bank on
  PE (I-matmul folds the state term), so the soft-threshold reads PSUM
  directly and two DVE adds disappear from the critical path.
- q = U^T v runs on PE: v itself is the matmul lhsT (M=32), giving
  R1[c, (m, c')] = sum_k v[k,c] U[k+128c', m]; the c==c' diagonal is
  pulled out with a partition-skewed access pattern and a ones-matmul
  reduces over partitions and broadcasts q in one shot.
- c0 = B^-1 bA rides along as a 10th rank-one column with q10 = -1, so
  no separate c0 matmul is needed.
- state tiles (x, v) are bf16; Tn/Dt/tATn/tAB on GpSimd, Cx/Wn/casts on
  Scalar, everything latency-critical on DVE/PE.

Vector layout: [128, 34] tiles with halo columns, flat index
i = k + 128*c stored at tile[:, c+1]; col 0 mirrors col 32 and col 33
mirrors col 1 (circular wrap).  Banded operators are a main [128,128]
matmul on cols 1:33 plus corner blocks on the shifted views.
"""

import numpy as np

N = 4096
P = 128          # partitions
CCOL = 32        # payload columns; i = k + 128*c at tile col c+1
HCOL = CCOL + 2  # halo tile width
RB = 32          # band radius of G
R9 = 9           # Woodbury rank
R10 = 10         # rank columns incl. the c0 slot
GAMMA = 10.0
ALPHA = 5.0
LAM = 1e-4
NIT = 50
NCORES = 8
USE_SKEW = False  # PE-based q with skewed diagonal AP (False: DVE Z1 path)

# f32-column offsets inside the constant blob [128, BLOB_COLS].
# bf16 payloads are packed two-per-f32-column and bitcast on device.
_cur = 0
def _alloc(w):
    global _cur
    off = _cur
    _cur += w
    return off

OFF_A1M   = _alloc(64)    # gamma*(S+ - I) main, bf16 [128,128]
OFF_BM    = _alloc(64)    # gamma*(S- - I) main
OFF_IBF   = _alloc(64)    # identity (tAB fold into the At bank)
OFF_A1C   = _alloc(64)    # A1 corner (single element, padded)
OFF_BC    = _alloc(64)    # B corner
OFF_GHIM  = _alloc(64)    # C^-1 banded main, hi half
OFF_GLOM  = _alloc(64)    # lo half
OFF_GBLH  = _alloc(64)    # C^-1 left corner hi
OFF_GBLL  = _alloc(64)
OFF_GBHH  = _alloc(64)    # C^-1 right corner hi
OFF_GBHL  = _alloc(64)
OFF_ONES  = _alloc(64)    # all-ones bf16 (partition reduce + broadcast)
OFF_U2    = _alloc(144)   # U m-major: [k, m*32+c], bf16 [128, 288]
OFF_W2    = _alloc(160)   # [W2 | c0] c-major: [k, c*10+m], bf16 [128, 320]
OFF_X0    = _alloc(17)    # x0 tile with halo, bf16 [128, 34]
BLOB_COLS = _cur


def _bf16(x):
    x32 = np.asarray(x, np.float32)
    u = x32.view(np.uint32)
    r = ((u >> 16) + ((u >> 15) & 1)).astype(np.uint32) << 16
    return r.view(np.float32)


def _pack_bf16(arr):
    """[128, W] float (W even) -> [128, W//2] f32 with packed bf16 pairs."""
    a = _bf16(arr).view(np.uint32) >> 16
    lo, hi = a[:, 0::2], a[:, 1::2]
    return (lo | (hi << 16)).view(np.float32)


def _banded(h):
    """main/BL/BH lhsT pieces for kernel h (dict d -> coef), [128,128] each.
    lhsT[k, m]: contraction index k = input row, m = output row.
    main: within-column (shift d = k - m);
    BL: rhs = col c-1 view (shift d = k - 128 - m);
    BH: rhs = col c+1 view (shift d = k + 128 - m)."""
    B0 = np.zeros((P, P)); BL = np.zeros((P, P)); BH = np.zeros((P, P))
    for k in range(P):
        for m in range(P):
            if (k - m) in h:
                B0[k, m] = h[k - m]
            if (k - P - m) in h:
                BL[k, m] = h[k - P - m]
            if (k + P - m) in h:
                BH[k, m] = h[k + P - m]
    return B0, BL, BH


def _tile(vec):
    """[4096] -> [128, 32], i = k + 128*c."""
    return np.ascontiguousarray(np.asarray(vec).reshape(CCOL, P).T)


def host_constants(target, A, x0):
    """All f64 precompute; returns the [128, BLOB_COLS] f32 device blob."""
    A64 = np.asarray(A, np.float64)
    w = ALPHA + 2 * GAMMA * (1 - np.cos(2 * np.pi * np.arange(N // 2 + 1) / N))

    def C_inv(z):
        return np.fft.irfft(np.fft.rfft(z, axis=-1) / w, n=N, axis=-1)

    U = C_inv(A64).T                              # [N, 9]
    S = np.eye(R9) + A64 @ U
    W2 = U @ np.linalg.inv(S)                     # [N, 9]
    g = np.fft.irfft(1.0 / w, n=N)                # kernel of C^-1
    b = A64 @ np.asarray(target, np.float64)
    bA = b @ A64
    c0 = C_inv(bA) - W2 @ (U.T @ bA)              # B^-1 @ bA

    blob = np.zeros((P, BLOB_COLS), np.float32)

    def putb(off, arr):
        p = _pack_bf16(arr)
        blob[:p.shape[0], off:off + p.shape[1]] = p

    mA1 = _banded({-1: GAMMA, 0: -GAMMA})         # gamma*(S+ - I)
    mB = _banded({1: GAMMA, 0: -GAMMA})           # gamma*(S- - I)
    mG = _banded({d: g[d % N] for d in range(-RB, RB + 1)})
    Ghi = [_bf16(m).astype(np.float64) for m in mG]
    Glo = [m - h for m, h in zip(mG, Ghi)]

    putb(OFF_A1M, mA1[0]); putb(OFF_A1C, mA1[1])
    putb(OFF_BM, mB[0]); putb(OFF_BC, mB[2])
    putb(OFF_GHIM, Ghi[0]); putb(OFF_GLOM, Glo[0])
    putb(OFF_GBLH, Ghi[1]); putb(OFF_GBLL, Glo[1])
    putb(OFF_GBHH, Ghi[2]); putb(OFF_GBHL, Glo[2])
    putb(OFF_IBF, np.eye(P))
    putb(OFF_ONES, np.ones((P, P)))

    # U2[k, m*32+c] = U[k + 128c, m] (m-major)
    putb(OFF_U2, U.reshape(CCOL, P, R9).transpose(1, 2, 0).reshape(P, R9 * CCOL))
    # W2e[k, c*10+m] = W2[k + 128c, m] for m<9;  c0[k + 128c] at m=9
    W2e = np.concatenate([W2, c0[:, None]], axis=1)     # [N, 10]
    putb(OFF_W2, W2e.reshape(CCOL, P, R10).transpose(1, 0, 2).reshape(P, CCOL * R10))

    x0t = _tile(np.asarray(x0, np.float64))
    x0h = np.zeros((P, HCOL))
    x0h[:, 1:CCOL + 1] = x0t
    x0h[:, 0] = x0t[:, CCOL - 1]
    x0h[:, CCOL + 1] = x0t[:, 0]
    putb(OFF_X0, x0h)
    return np.ascontiguousarray(blob)


def build_nc():
    """Build and compile the Bacc graph (one core's program)."""
    from concourse import bacc, bass, mybir, tile

    f32 = mybir.dt.float32
    bf16 = mybir.dt.bfloat16
    Relu = mybir.ActivationFunctionType.Relu
    Copy = mybir.ActivationFunctionType.Copy
    Alu = mybir.AluOpType
    nc = bacc.Bacc(target_bir_lowering=False)

    blob_ext = nc.declare_dram_parameter("blob", [P, BLOB_COLS], f32, isOutput=False)
    out_ext = nc.declare_dram_parameter("out", [P, CCOL], f32, isOutput=True)

    with tile.TileContext(nc) as tc:
        with (
            tc.tile_pool(name="const", bufs=1) as cpool,
            tc.tile_pool(name="work", bufs=3) as wpool,
            tc.tile_pool(name="psum", bufs=1, space="PSUM") as ppool,
        ):
            cb = cpool.tile([P, BLOB_COLS], f32, tag="blob")
            nc.sync.dma_start(cb[:, :], blob_ext[:, :])

            def csb(off, wcols):
                """bf16 view of wcols f32 columns -> [128, 2*wcols] bf16"""
                return cb[:, off:off + wcols].bitcast(bf16)

            A1_M, A1_C = csb(OFF_A1M, 64), csb(OFF_A1C, 64)
            B_M, B_C = csb(OFF_BM, 64), csb(OFF_BC, 64)
            I_bf = csb(OFF_IBF, 64)
            GHI_M, GLO_M = csb(OFF_GHIM, 64), csb(OFF_GLOM, 64)
            GBL_H, GBL_L = csb(OFF_GBLH, 64), csb(OFF_GBLL, 64)
            GBH_H, GBH_L = csb(OFF_GBHH, 64), csb(OFF_GBHL, 64)
            ones_bf = csb(OFF_ONES, 64)
            ones32 = cb[0:CCOL, OFF_ONES:OFF_ONES + 64].bitcast(bf16)
            U2 = csb(OFF_U2, 144)                  # [128, 288] bf16
            U2_3d = U2.rearrange("k (m c) -> k m c", c=CCOL)
            W2e = csb(OFF_W2, 160)                 # [128, 320] bf16
            W2_3d = W2e.rearrange("k (c m) -> k c m", m=R10)

            # persistent rank tile: cols 0:9 = diag partials (rewritten each
            # iter), col 9 = -1/32 so the ones-reduce yields q10 = -1 (c0 slot)
            qpart = cpool.tile([CCOL, 16], bf16, tag="qpart")
            nc.vector.memset(qpart[:, :], 0.0)
            nc.vector.memset(qpart[:, R9:R9 + 1], -1.0 / CCOL)

            Xh = csb(OFF_X0, 17)                   # [128, 34] bf16 state
            En_sb = Tn_sb = Un_sb = Wn_sb = None
            tABf = tABb = tTW = None

            for j in range(1, NIT + 1):
                first = (j == 1)
                last = (j == NIT)
                xm = Xh[:, 1:CCOL + 1]
                xl = Xh[:, 0:CCOL]
                xr = Xh[:, 2:HCOL]

                # --- PE: At bank (A1 x + B x + tAB) and B bank ---
                bankAt = ppool.tile([P, CCOL], f32, tag="At")
                bankB = ppool.tile([P, CCOL], f32, tag="B")
                if first:
                    nc.tensor.matmul(bankAt[:, :], A1_M, xm, start=True, stop=False)
                    nc.tensor.matmul(bankAt[:, :], A1_C, xl, start=False, stop=True)
                else:
                    nc.tensor.matmul(bankAt[:, :], I_bf, tABb[:, :], start=True, stop=False)
                    nc.tensor.matmul(bankAt[:, :], A1_M, xm, start=False, stop=False)
                    nc.tensor.matmul(bankAt[:, :], B_M, xm, start=False, stop=False)
                    nc.tensor.matmul(bankAt[:, :], A1_C, xl, start=False, stop=False)
                    nc.tensor.matmul(bankAt[:, :], B_C, xr, start=False, stop=True)
                nc.tensor.matmul(bankB[:, :], B_M, xm, start=True, stop=False)
                nc.tensor.matmul(bankB[:, :], B_C, xr, start=False, stop=True)

                # --- Scalar / GpSimd: alpha*x and the tau-side chain ---
                if first:
                    Wn = wpool.tile([P, CCOL], f32, tag="Wn")
                    nc.scalar.activation(Wn[:, :], xm, Relu, bias=0.0, scale=ALPHA)
                else:
                    Cx = wpool.tile([P, CCOL], f32, tag="Cx")
                    nc.scalar.activation(Cx[:, :], xm, Copy, bias=0.0, scale=ALPHA)
                    Tn = wpool.tile([P, CCOL], f32, tag="Tn")
                    Dt = wpool.tile([P, CCOL], f32, tag="Dt")
                    tATn = wpool.tile([P, CCOL], f32, tag="tATn")
                    nc.gpsimd.tensor_add(Tn[:, :], Cx[:, :], tTW[:, :])
                    nc.gpsimd.tensor_add(Dt[:, :], Cx[:, :], Tn[:, :])
                    nc.gpsimd.tensor_add(tATn[:, :], tABf[:, :], Tn[:, :])
                    Wn = wpool.tile([P, CCOL], f32, tag="Wn")
                    nc.scalar.activation(Wn[:, :], Dt[:, :], Relu, bias=0.0, scale=1.0)

                # --- DVE: soft-threshold and v ---
                r1 = wpool.tile([P, CCOL], f32, tag="r1")
                Un = wpool.tile([P, CCOL], f32, tag="Un")
                nc.vector.tensor_scalar(r1[:, :], bankAt[:, :], -LAM, LAM,
                                        Alu.max, Alu.min)
                nc.vector.tensor_sub(Un[:, :], bankAt[:, :], r1[:, :])

                vh = wpool.tile([P, HCOL], bf16, tag="vh")
                vm, vl, vr = vh[:, 1:CCOL + 1], vh[:, 0:CCOL], vh[:, 2:HCOL]
                if first:
                    nc.vector.tensor_add(vm, Un[:, :], Wn[:, :])
                else:
                    P1 = wpool.tile([P, CCOL], f32, tag="P1")
                    P2n = wpool.tile([P, CCOL], f32, tag="P2n")
                    # P1 = relu(Dt) + Un;  P2n = -bankB - tATn = -(En + Tn)
                    nc.vector.scalar_tensor_tensor(P1[:, :], Dt[:, :], 0.0,
                                                   Un[:, :], Alu.max, Alu.add)
                    nc.vector.scalar_tensor_tensor(P2n[:, :], bankB[:, :], -1.0,
                                                   tATn[:, :], Alu.mult,
                                                   Alu.subtract)
                    nc.vector.tensor_add(vm, P1[:, :], P2n[:, :])
                # v halo: col0 mirrors col32, col33 mirrors col1
                nc.vector.tensor_copy(vh[:, 0:1], vh[:, CCOL:CCOL + 1])
                nc.vector.tensor_copy(vh[:, HCOL - 1:HCOL], vh[:, 1:2])

                # --- PE: rank-9 q and the banded G apply ---
                bankE = ppool.tile([P, CCOL], f32, tag="E")
                bankR = ppool.tile([P, 16], f32, tag="R")
                if USE_SKEW:
                    R1 = ppool.tile([CCOL, R9 * CCOL], f32, tag="R1")
                    nc.tensor.matmul(R1[:, :], vm, U2, start=True, stop=True)
                nc.tensor.matmul(bankE[:, :], GHI_M, vm, start=True, stop=False)
                nc.tensor.matmul(bankE[:, :], GLO_M, vm, start=False, stop=False)
                if USE_SKEW:
                    # diag: qpart[c, m] = R1[c, m*32 + c]  (skewed AP, then
                    # ones-matmul = partition reduce + broadcast)
                    r1ap = R1[:, :]
                    diag = bass.AP(tensor=r1ap.tensor, offset=r1ap.offset,
                                   ap=[[r1ap.ap[0][0] + 1, CCOL], [CCOL, R9]])
                    nc.vector.tensor_copy(qpart[:, 0:R9], diag)
                    nc.tensor.matmul(bankR[:, 0:R10], ones32, qpart[:, 0:R10],
                                     start=True, stop=True)
                else:
                    Z1 = wpool.tile([P, R9 * CCOL], bf16, tag="Z1")
                    Z1r = wpool.tile([P, 16], f32, tag="Z1r")
                    Z1b = wpool.tile([P, 16], bf16, tag="Z1b")
                    z1_3d = Z1[:, :].rearrange("k (m c) -> k m c", c=CCOL)
                    vb9 = vm.unsqueeze(1).broadcast_to([P, R9, CCOL])
                    nc.vector.tensor_mul(z1_3d, U2_3d, vb9)
                    nc.vector.tensor_reduce(Z1r[:, 0:R9], z1_3d,
                                            axis=mybir.AxisListType.X, op=Alu.add)
                    nc.vector.memset(Z1r[:, R9:R9 + 1], -1.0 / P)
                    nc.vector.tensor_copy(Z1b[:, 0:R10], Z1r[:, 0:R10])
                    nc.tensor.matmul(bankR[:, 0:R10], ones_bf, Z1b[:, 0:R10],
                                     start=True, stop=True)
                nc.tensor.matmul(bankE[:, :], GBL_H, vl, start=False, stop=False)
                nc.tensor.matmul(bankE[:, :], GBL_L, vl, start=False, stop=False)
                nc.tensor.matmul(bankE[:, :], GBH_H, vr, start=False, stop=False)
                nc.tensor.matmul(bankE[:, :], GBH_L, vr, start=False, stop=True)

                # --- DVE: rank-9 correction and the x update ---
                qb = wpool.tile([P, 16], bf16, tag="qb")
                nc.vector.tensor_copy(qb[:, 0:R10], bankR[:, 0:R10])
                Z2 = wpool.tile([P, CCOL * R10], bf16, tag="Z2")
                corr = wpool.tile([P, CCOL], f32, tag="corr")
                z2_3d = Z2[:, :].rearrange("k (c m) -> k c m", m=R10)
                rb = qb[:, 0:R10].unsqueeze(1).broadcast_to([P, CCOL, R10])
                nc.vector.tensor_mul(z2_3d, W2_3d, rb)
                nc.vector.tensor_reduce(corr[:, :], z2_3d,
                                        axis=mybir.AxisListType.X, op=Alu.add)

                if not last:
                    if not first:
                        # En (state only; At already folded on PE)
                        En = wpool.tile([P, CCOL], f32, tag="En")
                        nc.vector.tensor_add(En[:, :], bankB[:, :], tABf[:, :])

                    Xn = wpool.tile([P, HCOL], bf16, tag="Xh")
                    nc.vector.tensor_sub(Xn[:, 1:CCOL + 1], bankE[:, :], corr[:, :])
                    # x halo: col0 mirrors col32, col33 mirrors col1
                    nc.vector.tensor_copy(Xn[:, 0:1], Xn[:, CCOL:CCOL + 1])
                    nc.vector.tensor_copy(Xn[:, HCOL - 1:HCOL], Xn[:, 1:2])
                    Xh = Xn

                    # next-iteration state combos (run in this iter's tail)
                    tABf = wpool.tile([P, CCOL], f32, tag="tABf")
                    tTW = wpool.tile([P, CCOL], f32, tag="tTW")
                    tABb = wpool.tile([P, CCOL], bf16, tag="tABb")
                    if first:
                        nc.gpsimd.tensor_scalar_mul(tABf[:, :], Un[:, :], -1.0)
                        nc.gpsimd.tensor_scalar_mul(tTW[:, :], Wn[:, :], -1.0)
                    else:
                        nc.gpsimd.tensor_sub(tABf[:, :], En[:, :], Un[:, :])
                        nc.gpsimd.tensor_sub(tTW[:, :], Tn[:, :], Wn[:, :])
                    nc.scalar.activation(tABb[:, :], tABf[:, :], Copy,
                                         bias=0.0, scale=1.0)
                else:
                    Xout = wpool.tile([P, CCOL], f32, tag="Xout")
                    nc.vector.tensor_sub(Xout[:, :], bankE[:, :], corr[:, :])
                    nc.sync.dma_start(out_ext[:, :], Xout[:, :])

    nc.compile()
    return nc


def kernel(**inputs):
    from concourse.bass_utils import run_bass_kernel_spmd

    target = np.asarray(inputs["target"], np.float32)
    A = np.asarray(inputs["A"], np.float32)
    x0 = np.asarray(inputs["x0"], np.float32)

    blob = host_constants(target, A, x0)
    nc = build_nc()
    in_maps = [{"blob": blob} for _ in range(NCORES)]
    res = run_bass_kernel_spmd(nc, in_maps, core_ids=list(range(NCORES)))
    out_tile = np.asarray(res.results[0]["out"], np.float32)
    return np.ascontiguousarray(out_tile.T.reshape(-1))
